# revision 16
# baseline (speedup 1.0000x reference)
"""Trainium2 Bass kernel for nn_Eye_Center (sparse_attention).

Sharding: 8 cores = (batch B=2) x (4 head-groups of 3 heads each).
Each core computes, for its (b, head-group):
  - gw path:  attn_weight[b, 3hg:3hg+3] = softmax(q k^T) on the pooled image
  - D path:   3 heads of full attention (N=1024) + partial projection
  - 4 small blocks (N=192): 3 heads of attention + partial projection
Host does: avgpool / slicing (pre), partial-proj sum over head groups,
bias add, bilinear upsample, zscore, scatter (post). All O(MB) numpy work.
"""

import numpy as np

# ---- problem constants (hardcoded per the task contract) ----
B = 2
C = 384
NH = 12
DH = 32
SCALE = DH ** -0.5
HPG = 3                  # heads per group
NG = 4                   # head groups
HG_C = HPG * DH          # 96 channels per head group
NP = 1024                # pooled tokens (32*32)
ND = 1024                # D-block tokens (32*32)
NS = 192                 # small-block tokens
P = 128

_CACHED = {}


def _build_nc():
    import concourse.bass as bass
    import concourse.tile as tile
    from concourse import bacc, mybir
    from concourse.masks import make_identity

    f32 = mybir.dt.float32
    bf16 = mybir.dt.bfloat16
    EXP = mybir.ActivationFunctionType.Exp

    nc = bacc.Bacc(None)

    # ---- per-core I/O ----
    xp_d = nc.dram_tensor("xp", (P, 3, NP), bf16, kind="ExternalInput")
    xd_d = nc.dram_tensor("xd", (P, 3, ND), bf16, kind="ExternalInput")
    xs_d = nc.dram_tensor("xs", (P, 3, 4 * NS), bf16, kind="ExternalInput")
    gwq_d = nc.dram_tensor("gwq", (P, 3, HG_C), bf16, kind="ExternalInput")
    gwk_d = nc.dram_tensor("gwk", (P, 3, HG_C), bf16, kind="ExternalInput")
    asq_d = nc.dram_tensor("asq", (P, 3, HG_C), bf16, kind="ExternalInput")
    ask_d = nc.dram_tensor("ask", (P, 3, HG_C), bf16, kind="ExternalInput")
    asv_d = nc.dram_tensor("asv", (P, 3, HG_C), bf16, kind="ExternalInput")
    wp_d = nc.dram_tensor("wp", (HG_C, C), bf16, kind="ExternalInput")
    bias_d = nc.dram_tensor("bias", (HG_C, 4), f32, kind="ExternalInput")
    bvrow_d = nc.dram_tensor("bvrow", (1, HG_C), bf16, kind="ExternalInput")

    attnw_d = nc.dram_tensor("attnw", (HPG, NP, NP), f32, kind="ExternalOutput")
    fpd_d = nc.dram_tensor("fpd", (P, 3, ND), f32, kind="ExternalOutput")
    fps_d = nc.dram_tensor("fps", (P, 3, 4 * NS), f32, kind="ExternalOutput")

    def mm(out, lhsT, rhs, **kw):
        nc.tensor.matmul(out, lhsT, rhs, **kw)

    with tile.TileContext(nc) as tc:
        with tc.tile_pool(name="const", bufs=1) as consts:
            ident = consts.tile([P, P], bf16)
            make_identity(nc, ident[:])
            ones_sb = consts.tile([1, P], bf16)
            nc.vector.memset(ones_sb[:], 1.0)

            xp_sb = consts.tile([P, 3, NP], bf16)
            nc.sync.dma_start(xp_sb[:], xp_d[:])
            xd_sb = consts.tile([P, 3, ND], bf16)
            nc.sync.dma_start(xd_sb[:], xd_d[:])
            xs_sb = consts.tile([P, 3, 4 * NS], bf16)
            nc.sync.dma_start(xs_sb[:], xs_d[:])
            gwq_sb = consts.tile([P, 3, HG_C], bf16)
            nc.sync.dma_start(gwq_sb[:], gwq_d[:])
            gwk_sb = consts.tile([P, 3, HG_C], bf16)
            nc.sync.dma_start(gwk_sb[:], gwk_d[:])
            asq_sb = consts.tile([P, 3, HG_C], bf16)
            nc.sync.dma_start(asq_sb[:], asq_d[:])
            ask_sb = consts.tile([P, 3, HG_C], bf16)
            nc.sync.dma_start(ask_sb[:], ask_d[:])
            asv_sb = consts.tile([P, 3, HG_C], bf16)
            nc.sync.dma_start(asv_sb[:], asv_d[:])
            wp_sb = consts.tile([HG_C, C], bf16)
            nc.sync.dma_start(wp_sb[:], wp_d[:])
            bias_sb = consts.tile([HG_C, 4], f32)
            nc.sync.dma_start(bias_sb[:], bias_d[:])
            bvrow_sb = consts.tile([1, HG_C], bf16)
            nc.sync.dma_start(bvrow_sb[:], bvrow_d[:])

            # staging tiles that persist across phases
            gw_qT = consts.tile([HG_C, NP], bf16)
            gw_kT = consts.tile([HG_C, NP], bf16)
            as_qT = consts.tile([HG_C, ND], bf16)
            as_kT = consts.tile([HG_C, ND], bf16)
            v1_sb = consts.tile([P, HPG, 8, DH + 1], bf16)   # D: [v_h | ones]
            nc.vector.memset(v1_sb[:], 1.0)
            ocn_sb = consts.tile([P, 8, HG_C], bf16)         # D: normalized out (n-part)
            outTn_sb = consts.tile([HG_C, ND], bf16)         # D: normalized out^T
            fpd_sb = consts.tile([P, 3, ND], f32)
            qTs = consts.tile([HG_C, 4 * NS], bf16)
            kTs = consts.tile([HG_C, 4 * NS], bf16)
            v1s_sb = consts.tile([HG_C, 4, 2, HPG, DH + 1], bf16)  # small v' per (blk, mchunk, h)
            nc.vector.memset(v1s_sb[:], 1.0)
            ocns_sb = consts.tile([HG_C, 4, 2, HG_C], bf16)
            outTsn_sb = consts.tile([HG_C, 4 * NS], bf16)
            fps_sb = consts.tile([P, 3, 4 * NS], f32)

            # ---------- phase 1: qkv projections (gw q/k, D q/k/v, small q/k/v) ----------
            with tc.tile_pool(name="qk_ps", bufs=2, space="PSUM") as pqk, \
                 tc.tile_pool(name="v_ps", bufs=2, space="PSUM") as pv:
                for (src, w_sb, b_col, dst, n_tok) in (
                    (xp_sb, gwq_sb, 0, gw_qT, NP),
                    (xp_sb, gwk_sb, 1, gw_kT, NP),
                    (xd_sb, asq_sb, 2, as_qT, ND),
                    (xd_sb, ask_sb, 3, as_kT, ND),
                    (xs_sb, asq_sb, 2, qTs, 4 * NS),
                    (xs_sb, ask_sb, 3, kTs, 4 * NS),
                ):
                    q_ps = pqk.tile([HG_C, n_tok], f32, tag="qk")
                    for (c0, cw) in ((0, 512), (512, n_tok - 512)):
                        for k in range(3):
                            mm(q_ps[:, c0:c0 + cw],
                               w_sb[:, k, :], src[:, k, c0:c0 + cw],
                               start=(k == 0), stop=(k == 2))
                    nc.vector.tensor_scalar_add(dst[:], q_ps[:], bias_sb[:, b_col:b_col + 1])

                # D-path v, natural layout (m on partitions), with bias via K=1 ones matmul
                for mc in range(8):
                    v_ps = pv.tile([P, HG_C], f32, tag="v")
                    for k in range(3):
                        mm(v_ps[:], xd_sb[:, k, mc * P:(mc + 1) * P], asv_sb[:, k, :],
                           start=(k == 0), stop=False)
                    mm(v_ps[:], ones_sb[:, 0:P], bvrow_sb[:], start=False, stop=True)
                    nc.vector.tensor_copy(
                        v1_sb[:, :, mc, 0:DH],
                        v_ps[:].rearrange("p (h d) -> p h d", h=HPG))

                # small-path v
                for j in range(4):
                    for mc in range(2):
                        vs_ps = pv.tile([96, HG_C], f32, tag="vs")
                        for k in range(3):
                            mm(vs_ps[:], xs_sb[:, k, j * NS + mc * 96: j * NS + (mc + 1) * 96],
                               asv_sb[:, k, :], start=(k == 0), stop=False)
                        mm(vs_ps[:], ones_sb[:, 0:96], bvrow_sb[:], start=False, stop=True)
                        nc.vector.tensor_copy(
                            v1s_sb[:96, j, mc, :, 0:DH],
                            vs_ps[:].rearrange("p (h d) -> p h d", h=HPG))

            # ---------- phase 2: gw attention -> attn_weight (softmax rows) ----------
            with tc.tile_pool(name="s_ps", bufs=4, space="PSUM") as ps, \
                 tc.tile_pool(name="p_sb", bufs=4) as pp, \
                 tc.tile_pool(name="r_sb", bufs=8) as pr:
                for nb in range(8):
                    for h in range(HPG):
                        hs = h * DH
                        s_ps = ps.tile([P, NP], f32, tag="s")
                        for half in range(2):
                            mm(s_ps[:, half * 512:(half + 1) * 512],
                               gw_qT[hs:hs + DH, nb * P:(nb + 1) * P],
                               gw_kT[hs:hs + DH, half * 512:(half + 1) * 512],
                               start=True, stop=True, tile_position=(hs, 0))
                        p_sb = pp.tile([P, NP], f32, tag="p")
                        r = pr.tile([P, 1], f32, tag="r")
                        nc.scalar.activation(p_sb[:], s_ps[:], EXP, accum_out=r[:])
                        rinv = pr.tile([P, 1], f32, tag="ri")
                        nc.vector.reciprocal(rinv[:], r[:])
                        nc.gpsimd.tensor_scalar_mul(p_sb[:], p_sb[:], rinv[:])
                        nc.sync.dma_start(attnw_d[h, nb * P:(nb + 1) * P, :], p_sb[:])

            # ---------- phase 3: D attention (S^T layout) + AV with ones column ----------
            with tc.tile_pool(name="o_sb", bufs=1) as posb:
                outT_sb = [posb.tile([DH + 1, ND], bf16, tag=f"osb{h}", name=f"osb{h}")
                           for h in range(HPG)]
                with tc.tile_pool(name="st_ps", bufs=4, space="PSUM") as pst, \
                     tc.tile_pool(name="ot_ps", bufs=1, space="PSUM") as pot, \
                     tc.tile_pool(name="pt_sb", bufs=6) as ppt:
                    for half in range(2):
                        outT = [pot.tile([DH + 1, 512], f32, tag=f"ot{h}", name=f"ot{h}")
                                for h in range(HPG)]
                        for mc in range(8):
                            for h in range(HPG):
                                hs = h * DH
                                st_ps = pst.tile([P, 512], f32, tag="st")
                                mm(st_ps[:], as_kT[hs:hs + DH, mc * P:(mc + 1) * P],
                                   as_qT[hs:hs + DH, half * 512:(half + 1) * 512],
                                   start=True, stop=True, tile_position=(hs, 0))
                                pt_sb = ppt.tile([P, 512], bf16, tag="pt")
                                nc.scalar.activation(pt_sb[:], st_ps[:], EXP)
                                mm(outT[h][:], v1_sb[:, h, mc, :], pt_sb[:],
                                   start=(mc == 0), stop=(mc == 7))
                        for h in range(HPG):
                            nc.vector.tensor_copy(
                                outT_sb[h][:, half * 512:(half + 1) * 512], outT[h][:])

                # transpose per (chunk, head), normalize by r (= row DH), collect n-part
                with tc.tile_pool(name="t_ps", bufs=4, space="PSUM") as ptr, \
                     tc.tile_pool(name="tb_ps", bufs=2, space="PSUM") as ptb, \
                     tc.tile_pool(name="ri2", bufs=8) as pri:
                    for cb in range(8):
                        for h in range(HPG):
                            t_ps = ptr.tile([P, DH + 1], bf16, tag="t")
                            nc.tensor.transpose(
                                t_ps[:], outT_sb[h][:, cb * P:(cb + 1) * P],
                                ident[0:DH + 1, 0:DH + 1])
                            rinv = pri.tile([P, 1], f32, tag="ri2")
                            nc.vector.reciprocal(rinv[:], t_ps[:, DH:DH + 1])
                            nc.vector.tensor_scalar_mul(
                                ocn_sb[:, cb, h * DH:(h + 1) * DH], t_ps[:, 0:DH], rinv[:])
                    # transpose back to (c_in, n)
                    for cb in range(8):
                        tb_ps = ptb.tile([HG_C, P], bf16, tag="tb")
                        nc.tensor.transpose(tb_ps[:], ocn_sb[:, cb, :], ident[:])
                        nc.vector.tensor_copy(outTn_sb[:, cb * P:(cb + 1) * P], tb_ps[:])

            # partial projection: fpd = wp[hg rows]^T @ outTn
            with tc.tile_pool(name="f_ps", bufs=2, space="PSUM") as pf:
                for co in range(3):
                    f_ps = pf.tile([P, ND], f32, tag="f")
                    for half in range(2):
                        mm(f_ps[:, half * 512:(half + 1) * 512],
                           wp_sb[:, co * P:(co + 1) * P],
                           outTn_sb[:, half * 512:(half + 1) * 512],
                           start=True, stop=True)
                    nc.vector.tensor_copy(fpd_sb[:, co, :], f_ps[:])
            nc.sync.dma_start(fpd_d[:], fpd_sb[:])

            # ---------- phase 4: small blocks ----------
            with tc.tile_pool(name="os_sb", bufs=12) as possb:
                outTs_sb = {}
                with tc.tile_pool(name="sts_ps", bufs=4, space="PSUM") as psts, \
                     tc.tile_pool(name="ots_ps", bufs=3, space="PSUM") as pots, \
                     tc.tile_pool(name="pts_sb", bufs=6) as ppts:
                    for j in range(4):
                        for h in range(HPG):
                            hs = h * DH
                            st_ps = psts.tile([96, 2, NS], f32, tag="sts")
                            for mc in range(2):
                                mm(st_ps[:, mc, :],
                                   kTs[hs:hs + DH, j * NS + mc * 96: j * NS + (mc + 1) * 96],
                                   qTs[hs:hs + DH, j * NS:(j + 1) * NS],
                                   start=True, stop=True, tile_position=(hs, 0))
                            pt_sb = ppts.tile([96, 2, NS], bf16, tag="pts")
                            nc.scalar.activation(pt_sb[:], st_ps[:], EXP)
                            ot_ps = pots.tile([DH + 1, NS], f32, tag="ots")
                            for mc in range(2):
                                mm(ot_ps[:], v1s_sb[:96, j, mc, h, :], pt_sb[:, mc, :],
                                   start=(mc == 0), stop=(mc == 1))
                            o_sb = possb.tile([DH + 1, NS], bf16, tag="oss")
                            nc.vector.tensor_copy(o_sb[:], ot_ps[:])
                            outTs_sb[(j, h)] = o_sb
                with tc.tile_pool(name="ts_ps", bufs=4, space="PSUM") as ptrs, \
                     tc.tile_pool(name="tbs_ps", bufs=2, space="PSUM") as ptbs, \
                     tc.tile_pool(name="ri3", bufs=8) as pri3:
                    for j in range(4):
                        for cb in range(2):
                            for h in range(HPG):
                                t_ps = ptrs.tile([96, DH + 1], bf16, tag="ts")
                                nc.tensor.transpose(
                                    t_ps[:], outTs_sb[(j, h)][:, cb * 96:(cb + 1) * 96],
                                    ident[0:DH + 1, 0:DH + 1])
                                rinv = pri3.tile([96, 1], f32, tag="ri3")
                                nc.vector.reciprocal(rinv[:], t_ps[:, DH:DH + 1])
                                nc.vector.tensor_scalar_mul(
                                    ocns_sb[:96, j, cb, h * DH:(h + 1) * DH],
                                    t_ps[:, 0:DH], rinv[:])
                        for cb in range(2):
                            tb_ps = ptbs.tile([HG_C, 96], bf16, tag="tbs")
                            nc.tensor.transpose(tb_ps[:], ocns_sb[:96, j, cb, :],
                                                ident[0:96, 0:96])
                            nc.vector.tensor_copy(
                                outTsn_sb[:, j * NS + cb * 96: j * NS + (cb + 1) * 96],
                                tb_ps[:])
            with tc.tile_pool(name="fs_ps", bufs=2, space="PSUM") as pfs:
                for co in range(3):
                    f_ps = pfs.tile([P, 4 * NS], f32, tag="fs")
                    for (c0, cw) in ((0, 512), (512, 256)):
                        mm(f_ps[:, c0:c0 + cw],
                           wp_sb[:, co * P:(co + 1) * P],
                           outTsn_sb[:, c0:c0 + cw],
                           start=True, stop=True)
                    nc.vector.tensor_copy(fps_sb[:, co, :], f_ps[:])
            nc.sync.dma_start(fps_d[:], fps_sb[:])

    nc.compile()
    return nc


# ---------------- host-side numpy helpers ----------------

def _avgpool2(x):
    b, c, h, w = x.shape
    return x.reshape(b, c, h // 2, 2, w // 2, 2).mean(axis=(3, 5))


def _up2_align_corners(x):
    # x: (C, H, W) -> (C, 2H, 2W), bilinear align_corners=True
    c, h, w = x.shape
    hn, wn = 2 * h, 2 * w

    def coords(n_out, n_in):
        s = np.arange(n_out) * ((n_in - 1) / (n_out - 1))
        i0 = np.floor(s).astype(np.int32)
        i1 = np.minimum(i0 + 1, n_in - 1)
        wt = (s - i0).astype(np.float32)
        return i0, i1, wt

    y0, y1, wy = coords(hn, h)
    x0, x1, wx = coords(wn, w)
    row = x[:, y0, :] * (1.0 - wy)[None, :, None] + x[:, y1, :] * wy[None, :, None]
    return row[:, :, x0] * (1.0 - wx) + row[:, :, x1] * wx


def _zscore(y):
    # y: (C, H, W); per-column stats over C*H (population std)
    c, h, w = y.shape
    f = y.reshape(c * h, w)
    mu = f.mean(axis=0)
    sd = f.std(axis=0)
    sd = np.where(sd == 0, 1.0, sd).astype(np.float32)
    return ((f - mu) / sd).reshape(c, h, w).astype(np.float32)


def _bf16(a):
    import ml_dtypes
    return np.ascontiguousarray(a).astype(ml_dtypes.bfloat16)


def _chunk3(a):
    # (384, n) -> (128, 3, n) with row = ko*128 + p, cast to bf16
    n = a.shape[1]
    return _bf16(a.reshape(3, P, n).transpose(1, 0, 2))


def _prep_inputs(x, gw_Wqkv, gw_bqkv, as_Wqkv, as_bqkv, as_Wproj):
    x = np.asarray(x, dtype=np.float32)
    xc = np.ascontiguousarray(x.transpose(0, 3, 1, 2))           # (B, C, 64, 64)
    x0 = _avgpool2(xc)                                           # (B, C, 32, 32)
    xpT = x0.reshape(B, C, NP)                                   # == xp^T per batch
    xdT = np.ascontiguousarray(xc[:, :, 16:48, 16:48]).reshape(B, C, ND)
    blocks = [x0[:, :, 0:8, 0:24], x0[:, :, 24:32, 8:32],
              x0[:, :, 8:32, 0:8], x0[:, :, 0:24, 24:32]]
    xsT = np.concatenate(
        [np.ascontiguousarray(blk).reshape(B, C, NS) for blk in blocks], axis=2)

    gw_Wqkv = np.asarray(gw_Wqkv, np.float32)
    as_Wqkv = np.asarray(as_Wqkv, np.float32)
    gw_bqkv = np.asarray(gw_bqkv, np.float32)
    as_bqkv = np.asarray(as_bqkv, np.float32)
    as_Wproj = np.asarray(as_Wproj, np.float32)

    in_maps = []
    for core in range(8):
        b, hg = divmod(core, 4)
        cs = slice(hg * HG_C, (hg + 1) * HG_C)
        csa = np.arange(hg * HG_C, (hg + 1) * HG_C)
        m = {
            "xp": _chunk3(xpT[b]),
            "xd": _chunk3(xdT[b]),
            "xs": _chunk3(xsT[b]),
            "gwq": _chunk3(gw_Wqkv[:, csa] * SCALE),
            "gwk": _chunk3(gw_Wqkv[:, C + csa]),
            "asq": _chunk3(as_Wqkv[:, csa] * SCALE),
            "ask": _chunk3(as_Wqkv[:, C + csa]),
            "asv": _chunk3(as_Wqkv[:, 2 * C + csa]),
            "wp": _bf16(as_Wproj[cs, :]),
            "bias": np.ascontiguousarray(np.stack(
                [gw_bqkv[csa] * SCALE, gw_bqkv[C + csa],
                 as_bqkv[csa] * SCALE, as_bqkv[C + csa]], axis=1)),
            "bvrow": _bf16(as_bqkv[2 * C + csa][None, :]),
        }
        in_maps.append(m)
    return in_maps


def _postprocess(results, as_bproj):
    bp = np.asarray(as_bproj, np.float32)
    attn_weight = np.empty((B, NH, NP, NP), np.float32)
    attn_all = np.zeros((B, C, 64, 64), np.float32)
    for b in range(B):
        fpd = np.zeros((C, ND), np.float32)
        fps = np.zeros((C, 4 * NS), np.float32)
        for hg in range(NG):
            r = results[b * 4 + hg]
            attn_weight[b, hg * HPG:(hg + 1) * HPG] = r["attnw"]
            fpd += r["fpd"].transpose(1, 0, 2).reshape(C, ND)
            fps += r["fps"].transpose(1, 0, 2).reshape(C, 4 * NS)
        fpd += bp[:, None]
        fps += bp[:, None]
        aD = _zscore(fpd.reshape(C, 32, 32))
        small = []
        shapes = [(8, 24), (8, 24), (24, 8), (24, 8)]
        for j, (hh, ww) in enumerate(shapes):
            blk = fps[:, j * NS:(j + 1) * NS].reshape(C, hh, ww)
            small.append(_zscore(_up2_align_corners(blk)))
        aB1, aB2, aC1, aC2 = small
        attn_all[b, :, 0:16, 0:48] = aB1
        attn_all[b, :, 48:64, 16:64] = aB2
        attn_all[b, :, 16:64, 0:16] = aC1
        attn_all[b, :, 0:48, 48:64] = aC2
        attn_all[b, :, 16:48, 16:48] = aD
    return np.ascontiguousarray(attn_all.transpose(0, 2, 3, 1)), attn_weight


def _run(in_maps, trace=False, **kw):
    from concourse.bass_utils import run_bass_kernel_spmd
    if "nc" not in _CACHED:
        _CACHED["nc"] = _build_nc()
    return run_bass_kernel_spmd(_CACHED["nc"], in_maps, list(range(8)),
                                trace=trace, **kw)


def kernel(x, Cam_Centre, gw_Wqkv, gw_bqkv, as_Wqkv, as_bqkv, as_Wproj, as_bproj):
    in_maps = _prep_inputs(x, gw_Wqkv, gw_bqkv, as_Wqkv, as_bqkv, as_Wproj)
    res = _run(in_maps)
    return _postprocess(res.results, as_bproj)


# revision 17
# speedup vs baseline: 2.8679x; 2.8679x over previous
"""Trainium2 Bass kernel for nn_Eye_Center (sparse_attention).

Sharding: 8 cores = (batch B=2) x (4 head-groups of 3 heads each).
Each core computes, for its (b, head-group):
  - gw path:  attn_weight[b, 3hg:3hg+3] = softmax(q k^T) on the pooled image
  - D path:   3 heads of full attention (N=1024) + partial projection
  - 4 small blocks (N=192): 3 heads of attention + partial projection
Host does: avgpool / slicing (pre), partial-proj sum over head groups,
bias add, bilinear upsample, zscore, scatter (post). All O(MB) numpy work.
"""

import numpy as np

# ---- problem constants (hardcoded per the task contract) ----
B = 2
C = 384
NH = 12
DH = 32
SCALE = DH ** -0.5
HPG = 3                  # heads per group
NG = 4                   # head groups
HG_C = HPG * DH          # 96 channels per head group
NP = 1024                # pooled tokens (32*32)
ND = 1024                # D-block tokens (32*32)
NS = 192                 # small-block tokens
P = 128

_CACHED = {}


def _build_nc():
    import concourse.bass as bass
    import concourse.tile as tile
    from concourse import bacc, mybir
    from concourse.masks import make_identity

    f32 = mybir.dt.float32
    bf16 = mybir.dt.bfloat16
    EXP = mybir.ActivationFunctionType.Exp

    nc = bacc.Bacc(None)

    # ---- per-core I/O ----
    xp_d = nc.dram_tensor("xp", (P, 3, NP), bf16, kind="ExternalInput")
    xd_d = nc.dram_tensor("xd", (P, 3, ND), bf16, kind="ExternalInput")
    xs_d = nc.dram_tensor("xs", (P, 3, 4 * NS), bf16, kind="ExternalInput")
    gwq_d = nc.dram_tensor("gwq", (P, 3, HG_C), bf16, kind="ExternalInput")
    gwk_d = nc.dram_tensor("gwk", (P, 3, HG_C), bf16, kind="ExternalInput")
    asq_d = nc.dram_tensor("asq", (P, 3, HG_C), bf16, kind="ExternalInput")
    ask_d = nc.dram_tensor("ask", (P, 3, HG_C), bf16, kind="ExternalInput")
    asv_d = nc.dram_tensor("asv", (P, 3, HG_C), bf16, kind="ExternalInput")
    wp_d = nc.dram_tensor("wp", (HG_C, C), bf16, kind="ExternalInput")
    bias_d = nc.dram_tensor("bias", (HG_C, 4), f32, kind="ExternalInput")
    bvrow_d = nc.dram_tensor("bvrow", (1, HG_C), bf16, kind="ExternalInput")

    attnw_d = nc.dram_tensor("attnw", (HPG, NP, NP), f32, kind="ExternalOutput")
    fpd_d = nc.dram_tensor("fpd", (P, 3, ND), f32, kind="ExternalOutput")
    fps_d = nc.dram_tensor("fps", (P, 3, 4 * NS), f32, kind="ExternalOutput")

    def mm(out, lhsT, rhs, **kw):
        nc.tensor.matmul(out, lhsT, rhs, **kw)

    with tile.TileContext(nc) as tc:
        with tc.tile_pool(name="const", bufs=1) as consts:
            ident = consts.tile([P, P], bf16)
            make_identity(nc, ident[:])
            ones_sb = consts.tile([1, P], bf16)
            nc.vector.memset(ones_sb[:], 1.0)

            xp_sb = consts.tile([P, 3, NP], bf16)
            nc.sync.dma_start(xp_sb[:], xp_d[:])
            xd_sb = consts.tile([P, 3, ND], bf16)
            nc.sync.dma_start(xd_sb[:], xd_d[:])
            xs_sb = consts.tile([P, 3, 4 * NS], bf16)
            nc.sync.dma_start(xs_sb[:], xs_d[:])
            gwq_sb = consts.tile([P, 3, HG_C], bf16)
            nc.sync.dma_start(gwq_sb[:], gwq_d[:])
            gwk_sb = consts.tile([P, 3, HG_C], bf16)
            nc.sync.dma_start(gwk_sb[:], gwk_d[:])
            asq_sb = consts.tile([P, 3, HG_C], bf16)
            nc.sync.dma_start(asq_sb[:], asq_d[:])
            ask_sb = consts.tile([P, 3, HG_C], bf16)
            nc.sync.dma_start(ask_sb[:], ask_d[:])
            asv_sb = consts.tile([P, 3, HG_C], bf16)
            nc.sync.dma_start(asv_sb[:], asv_d[:])
            wp_sb = consts.tile([HG_C, C], bf16)
            nc.sync.dma_start(wp_sb[:], wp_d[:])
            bias_sb = consts.tile([HG_C, 4], f32)
            nc.sync.dma_start(bias_sb[:], bias_d[:])
            bvrow_sb = consts.tile([1, HG_C], bf16)
            nc.sync.dma_start(bvrow_sb[:], bvrow_d[:])

            # staging tiles that persist across phases
            gw_qT = consts.tile([HG_C, NP], bf16)
            gw_kT = consts.tile([HG_C, NP], bf16)
            as_qT = consts.tile([HG_C, ND], bf16)
            as_kT = consts.tile([HG_C, ND], bf16)
            v1_sb = consts.tile([P, HPG, 8, DH + 1], bf16)   # D: [v_h | ones]
            nc.vector.memset(v1_sb[:], 1.0)
            ocn_sb = consts.tile([P, 8, HG_C], bf16)         # D: normalized out (n-part)
            outTn_sb = consts.tile([HG_C, ND], bf16)         # D: normalized out^T
            fpd_sb = consts.tile([P, 3, ND], f32)
            qTs = consts.tile([HG_C, 4 * NS], bf16)
            kTs = consts.tile([HG_C, 4 * NS], bf16)
            v1s_sb = consts.tile([HG_C, 4, 2, HPG, DH + 1], bf16)  # small v' per (blk, mchunk, h)
            nc.vector.memset(v1s_sb[:], 1.0)
            ocns_sb = consts.tile([HG_C, 4, 2, HG_C], bf16)
            outTsn_sb = consts.tile([HG_C, 4 * NS], bf16)
            fps_sb = consts.tile([P, 3, 4 * NS], f32)

            # ---------- phase 1: qkv projections (gw q/k, D q/k/v, small q/k/v) ----------
            with tc.tile_pool(name="qk_ps", bufs=2, space="PSUM") as pqk, \
                 tc.tile_pool(name="v_ps", bufs=2, space="PSUM") as pv:
                for (src, w_sb, b_col, dst, n_tok) in (
                    (xp_sb, gwq_sb, 0, gw_qT, NP),
                    (xp_sb, gwk_sb, 1, gw_kT, NP),
                    (xd_sb, asq_sb, 2, as_qT, ND),
                    (xd_sb, ask_sb, 3, as_kT, ND),
                    (xs_sb, asq_sb, 2, qTs, 4 * NS),
                    (xs_sb, ask_sb, 3, kTs, 4 * NS),
                ):
                    q_ps = pqk.tile([HG_C, n_tok], f32, tag="qk")
                    for (c0, cw) in ((0, 512), (512, n_tok - 512)):
                        for k in range(3):
                            mm(q_ps[:, c0:c0 + cw],
                               w_sb[:, k, :], src[:, k, c0:c0 + cw],
                               start=(k == 0), stop=(k == 2))
                    nc.vector.tensor_scalar_add(dst[:], q_ps[:], bias_sb[:, b_col:b_col + 1])

                # D-path v, natural layout (m on partitions), with bias via K=1 ones matmul
                for mc in range(8):
                    v_ps = pv.tile([P, HG_C], f32, tag="v")
                    for k in range(3):
                        mm(v_ps[:], xd_sb[:, k, mc * P:(mc + 1) * P], asv_sb[:, k, :],
                           start=(k == 0), stop=False)
                    mm(v_ps[:], ones_sb[:, 0:P], bvrow_sb[:], start=False, stop=True)
                    nc.vector.tensor_copy(
                        v1_sb[:, :, mc, 0:DH],
                        v_ps[:].rearrange("p (h d) -> p h d", h=HPG))

                # small-path v
                for j in range(4):
                    for mc in range(2):
                        vs_ps = pv.tile([96, HG_C], f32, tag="vs")
                        for k in range(3):
                            mm(vs_ps[:], xs_sb[:, k, j * NS + mc * 96: j * NS + (mc + 1) * 96],
                               asv_sb[:, k, :], start=(k == 0), stop=False)
                        mm(vs_ps[:], ones_sb[:, 0:96], bvrow_sb[:], start=False, stop=True)
                        nc.vector.tensor_copy(
                            v1s_sb[:96, j, mc, :, 0:DH],
                            vs_ps[:].rearrange("p (h d) -> p h d", h=HPG))

            # ---------- phase 2: gw attention -> attn_weight (softmax rows) ----------
            with tc.tile_pool(name="s_ps", bufs=4, space="PSUM") as ps, \
                 tc.tile_pool(name="p_sb", bufs=4) as pp, \
                 tc.tile_pool(name="r_sb", bufs=8) as pr:
                for nb in range(8):
                    for h in range(HPG):
                        hs = h * DH
                        s_ps = ps.tile([P, NP], f32, tag="s")
                        for half in range(2):
                            mm(s_ps[:, half * 512:(half + 1) * 512],
                               gw_qT[hs:hs + DH, nb * P:(nb + 1) * P],
                               gw_kT[hs:hs + DH, half * 512:(half + 1) * 512],
                               start=True, stop=True, tile_position=(hs, 0))
                        p_sb = pp.tile([P, NP], f32, tag="p")
                        r = pr.tile([P, 1], f32, tag="r")
                        nc.scalar.activation(p_sb[:], s_ps[:], EXP, accum_out=r[:])
                        rinv = pr.tile([P, 1], f32, tag="ri")
                        nc.vector.reciprocal(rinv[:], r[:])
                        nc.vector.tensor_scalar_mul(p_sb[:], p_sb[:], rinv[:])
                        nc.sync.dma_start(attnw_d[h, nb * P:(nb + 1) * P, :], p_sb[:])

            # ---------- phase 3: D attention (S^T layout) + AV with ones column ----------
            with tc.tile_pool(name="o_sb", bufs=1) as posb:
                outT_sb = [posb.tile([DH + 1, ND], bf16, tag=f"osb{h}", name=f"osb{h}")
                           for h in range(HPG)]
                with tc.tile_pool(name="st_ps", bufs=4, space="PSUM") as pst, \
                     tc.tile_pool(name="ot_ps", bufs=1, space="PSUM") as pot, \
                     tc.tile_pool(name="pt_sb", bufs=6) as ppt:
                    for half in range(2):
                        outT = [pot.tile([DH + 1, 512], f32, tag=f"ot{h}", name=f"ot{h}")
                                for h in range(HPG)]
                        for mc in range(8):
                            for h in range(HPG):
                                hs = h * DH
                                st_ps = pst.tile([P, 512], f32, tag="st")
                                mm(st_ps[:], as_kT[hs:hs + DH, mc * P:(mc + 1) * P],
                                   as_qT[hs:hs + DH, half * 512:(half + 1) * 512],
                                   start=True, stop=True, tile_position=(hs, 0))
                                pt_sb = ppt.tile([P, 512], bf16, tag="pt")
                                nc.scalar.activation(pt_sb[:], st_ps[:], EXP)
                                mm(outT[h][:], v1_sb[:, h, mc, :], pt_sb[:],
                                   start=(mc == 0), stop=(mc == 7))
                        for h in range(HPG):
                            nc.vector.tensor_copy(
                                outT_sb[h][:, half * 512:(half + 1) * 512], outT[h][:])

                # transpose per (chunk, head), normalize by r (= row DH), collect n-part
                with tc.tile_pool(name="t_ps", bufs=4, space="PSUM") as ptr, \
                     tc.tile_pool(name="tb_ps", bufs=2, space="PSUM") as ptb, \
                     tc.tile_pool(name="ri2", bufs=8) as pri:
                    for cb in range(8):
                        for h in range(HPG):
                            t_ps = ptr.tile([P, DH + 1], bf16, tag="t")
                            nc.tensor.transpose(
                                t_ps[:], outT_sb[h][:, cb * P:(cb + 1) * P],
                                ident[0:DH + 1, 0:DH + 1])
                            rinv = pri.tile([P, 1], f32, tag="ri2")
                            nc.vector.reciprocal(rinv[:], t_ps[:, DH:DH + 1])
                            nc.vector.tensor_scalar_mul(
                                ocn_sb[:, cb, h * DH:(h + 1) * DH], t_ps[:, 0:DH], rinv[:])
                    # transpose back to (c_in, n)
                    for cb in range(8):
                        tb_ps = ptb.tile([HG_C, P], bf16, tag="tb")
                        nc.tensor.transpose(tb_ps[:], ocn_sb[:, cb, :], ident[:])
                        nc.vector.tensor_copy(outTn_sb[:, cb * P:(cb + 1) * P], tb_ps[:])

            # partial projection: fpd = wp[hg rows]^T @ outTn
            with tc.tile_pool(name="f_ps", bufs=2, space="PSUM") as pf:
                for co in range(3):
                    f_ps = pf.tile([P, ND], f32, tag="f")
                    for half in range(2):
                        mm(f_ps[:, half * 512:(half + 1) * 512],
                           wp_sb[:, co * P:(co + 1) * P],
                           outTn_sb[:, half * 512:(half + 1) * 512],
                           start=True, stop=True)
                    nc.vector.tensor_copy(fpd_sb[:, co, :], f_ps[:])
            nc.sync.dma_start(fpd_d[:], fpd_sb[:])

            # ---------- phase 4: small blocks ----------
            with tc.tile_pool(name="os_sb", bufs=12) as possb:
                outTs_sb = {}
                with tc.tile_pool(name="sts_ps", bufs=4, space="PSUM") as psts, \
                     tc.tile_pool(name="ots_ps", bufs=3, space="PSUM") as pots, \
                     tc.tile_pool(name="pts_sb", bufs=6) as ppts:
                    for j in range(4):
                        for h in range(HPG):
                            hs = h * DH
                            st_ps = psts.tile([96, 2, NS], f32, tag="sts")
                            for mc in range(2):
                                mm(st_ps[:, mc, :],
                                   kTs[hs:hs + DH, j * NS + mc * 96: j * NS + (mc + 1) * 96],
                                   qTs[hs:hs + DH, j * NS:(j + 1) * NS],
                                   start=True, stop=True, tile_position=(hs, 0))
                            pt_sb = ppts.tile([96, 2, NS], bf16, tag="pts")
                            nc.scalar.activation(pt_sb[:], st_ps[:], EXP)
                            ot_ps = pots.tile([DH + 1, NS], f32, tag="ots")
                            for mc in range(2):
                                mm(ot_ps[:], v1s_sb[:96, j, mc, h, :], pt_sb[:, mc, :],
                                   start=(mc == 0), stop=(mc == 1))
                            o_sb = possb.tile([DH + 1, NS], bf16, tag="oss")
                            nc.vector.tensor_copy(o_sb[:], ot_ps[:])
                            outTs_sb[(j, h)] = o_sb
                with tc.tile_pool(name="ts_ps", bufs=4, space="PSUM") as ptrs, \
                     tc.tile_pool(name="tbs_ps", bufs=2, space="PSUM") as ptbs, \
                     tc.tile_pool(name="ri3", bufs=8) as pri3:
                    for j in range(4):
                        for cb in range(2):
                            for h in range(HPG):
                                t_ps = ptrs.tile([96, DH + 1], bf16, tag="ts")
                                nc.tensor.transpose(
                                    t_ps[:], outTs_sb[(j, h)][:, cb * 96:(cb + 1) * 96],
                                    ident[0:DH + 1, 0:DH + 1])
                                rinv = pri3.tile([96, 1], f32, tag="ri3")
                                nc.vector.reciprocal(rinv[:], t_ps[:, DH:DH + 1])
                                nc.vector.tensor_scalar_mul(
                                    ocns_sb[:96, j, cb, h * DH:(h + 1) * DH],
                                    t_ps[:, 0:DH], rinv[:])
                        for cb in range(2):
                            tb_ps = ptbs.tile([HG_C, 96], bf16, tag="tbs")
                            nc.tensor.transpose(tb_ps[:], ocns_sb[:96, j, cb, :],
                                                ident[0:96, 0:96])
                            nc.vector.tensor_copy(
                                outTsn_sb[:, j * NS + cb * 96: j * NS + (cb + 1) * 96],
                                tb_ps[:])
            with tc.tile_pool(name="fs_ps", bufs=2, space="PSUM") as pfs:
                for co in range(3):
                    f_ps = pfs.tile([P, 4 * NS], f32, tag="fs")
                    for (c0, cw) in ((0, 512), (512, 256)):
                        mm(f_ps[:, c0:c0 + cw],
                           wp_sb[:, co * P:(co + 1) * P],
                           outTsn_sb[:, c0:c0 + cw],
                           start=True, stop=True)
                    nc.vector.tensor_copy(fps_sb[:, co, :], f_ps[:])
            nc.sync.dma_start(fps_d[:], fps_sb[:])

    nc.compile()
    return nc


# ---------------- host-side numpy helpers ----------------

def _avgpool2(x):
    b, c, h, w = x.shape
    return x.reshape(b, c, h // 2, 2, w // 2, 2).mean(axis=(3, 5))


def _up2_align_corners(x):
    # x: (C, H, W) -> (C, 2H, 2W), bilinear align_corners=True
    c, h, w = x.shape
    hn, wn = 2 * h, 2 * w

    def coords(n_out, n_in):
        s = np.arange(n_out) * ((n_in - 1) / (n_out - 1))
        i0 = np.floor(s).astype(np.int32)
        i1 = np.minimum(i0 + 1, n_in - 1)
        wt = (s - i0).astype(np.float32)
        return i0, i1, wt

    y0, y1, wy = coords(hn, h)
    x0, x1, wx = coords(wn, w)
    row = x[:, y0, :] * (1.0 - wy)[None, :, None] + x[:, y1, :] * wy[None, :, None]
    return row[:, :, x0] * (1.0 - wx) + row[:, :, x1] * wx


def _zscore(y):
    # y: (C, H, W); per-column stats over C*H (population std)
    c, h, w = y.shape
    f = y.reshape(c * h, w)
    mu = f.mean(axis=0)
    sd = f.std(axis=0)
    sd = np.where(sd == 0, 1.0, sd).astype(np.float32)
    return ((f - mu) / sd).reshape(c, h, w).astype(np.float32)


def _bf16(a):
    import ml_dtypes
    return np.ascontiguousarray(a).astype(ml_dtypes.bfloat16)


def _chunk3(a):
    # (384, n) -> (128, 3, n) with row = ko*128 + p, cast to bf16
    n = a.shape[1]
    return _bf16(a.reshape(3, P, n).transpose(1, 0, 2))


def _prep_inputs(x, gw_Wqkv, gw_bqkv, as_Wqkv, as_bqkv, as_Wproj):
    x = np.asarray(x, dtype=np.float32)
    xc = np.ascontiguousarray(x.transpose(0, 3, 1, 2))           # (B, C, 64, 64)
    x0 = _avgpool2(xc)                                           # (B, C, 32, 32)
    xpT = x0.reshape(B, C, NP)                                   # == xp^T per batch
    xdT = np.ascontiguousarray(xc[:, :, 16:48, 16:48]).reshape(B, C, ND)
    blocks = [x0[:, :, 0:8, 0:24], x0[:, :, 24:32, 8:32],
              x0[:, :, 8:32, 0:8], x0[:, :, 0:24, 24:32]]
    xsT = np.concatenate(
        [np.ascontiguousarray(blk).reshape(B, C, NS) for blk in blocks], axis=2)

    gw_Wqkv = np.asarray(gw_Wqkv, np.float32)
    as_Wqkv = np.asarray(as_Wqkv, np.float32)
    gw_bqkv = np.asarray(gw_bqkv, np.float32)
    as_bqkv = np.asarray(as_bqkv, np.float32)
    as_Wproj = np.asarray(as_Wproj, np.float32)

    in_maps = []
    for core in range(8):
        b, hg = divmod(core, 4)
        cs = slice(hg * HG_C, (hg + 1) * HG_C)
        csa = np.arange(hg * HG_C, (hg + 1) * HG_C)
        m = {
            "xp": _chunk3(xpT[b]),
            "xd": _chunk3(xdT[b]),
            "xs": _chunk3(xsT[b]),
            "gwq": _chunk3(gw_Wqkv[:, csa] * SCALE),
            "gwk": _chunk3(gw_Wqkv[:, C + csa]),
            "asq": _chunk3(as_Wqkv[:, csa] * SCALE),
            "ask": _chunk3(as_Wqkv[:, C + csa]),
            "asv": _chunk3(as_Wqkv[:, 2 * C + csa]),
            "wp": _bf16(as_Wproj[cs, :]),
            "bias": np.ascontiguousarray(np.stack(
                [gw_bqkv[csa] * SCALE, gw_bqkv[C + csa],
                 as_bqkv[csa] * SCALE, as_bqkv[C + csa]], axis=1)),
            "bvrow": _bf16(as_bqkv[2 * C + csa][None, :]),
        }
        in_maps.append(m)
    return in_maps


def _postprocess(results, as_bproj):
    bp = np.asarray(as_bproj, np.float32)
    attn_weight = np.empty((B, NH, NP, NP), np.float32)
    attn_all = np.zeros((B, C, 64, 64), np.float32)
    for b in range(B):
        fpd = np.zeros((C, ND), np.float32)
        fps = np.zeros((C, 4 * NS), np.float32)
        for hg in range(NG):
            r = results[b * 4 + hg]
            attn_weight[b, hg * HPG:(hg + 1) * HPG] = r["attnw"]
            fpd += r["fpd"].transpose(1, 0, 2).reshape(C, ND)
            fps += r["fps"].transpose(1, 0, 2).reshape(C, 4 * NS)
        fpd += bp[:, None]
        fps += bp[:, None]
        aD = _zscore(fpd.reshape(C, 32, 32))
        small = []
        shapes = [(8, 24), (8, 24), (24, 8), (24, 8)]
        for j, (hh, ww) in enumerate(shapes):
            blk = fps[:, j * NS:(j + 1) * NS].reshape(C, hh, ww)
            small.append(_zscore(_up2_align_corners(blk)))
        aB1, aB2, aC1, aC2 = small
        attn_all[b, :, 0:16, 0:48] = aB1
        attn_all[b, :, 48:64, 16:64] = aB2
        attn_all[b, :, 16:64, 0:16] = aC1
        attn_all[b, :, 0:48, 48:64] = aC2
        attn_all[b, :, 16:48, 16:48] = aD
    return np.ascontiguousarray(attn_all.transpose(0, 2, 3, 1)), attn_weight


def _run(in_maps, trace=False, **kw):
    from concourse.bass_utils import run_bass_kernel_spmd
    if "nc" not in _CACHED:
        _CACHED["nc"] = _build_nc()
    return run_bass_kernel_spmd(_CACHED["nc"], in_maps, list(range(8)),
                                trace=trace, **kw)


def kernel(x, Cam_Centre, gw_Wqkv, gw_bqkv, as_Wqkv, as_bqkv, as_Wproj, as_bproj):
    in_maps = _prep_inputs(x, gw_Wqkv, gw_bqkv, as_Wqkv, as_bqkv, as_Wproj)
    res = _run(in_maps)
    return _postprocess(res.results, as_bproj)


# revision 19
# speedup vs baseline: 3.0794x; 1.0738x over previous
"""Trainium2 Bass kernel for nn_Eye_Center (sparse_attention).

Sharding: 8 cores = (batch B=2) x (4 head-groups of 3 heads each).
Each core computes, for its (b, head-group):
  - gw path:  attn_weight[b, 3hg:3hg+3] = softmax(q k^T) on the pooled image
  - D path:   3 heads of full attention (N=1024) + partial projection
  - 4 small blocks (N=192): 3 heads of attention + partial projection
Host does: avgpool / slicing (pre), partial-proj sum over head groups,
bias add, bilinear upsample, zscore, scatter (post). All O(MB) numpy work.
"""

import numpy as np

# ---- problem constants (hardcoded per the task contract) ----
B = 2
C = 384
NH = 12
DH = 32
SCALE = DH ** -0.5
HPG = 3                  # heads per group
NG = 4                   # head groups
HG_C = HPG * DH          # 96 channels per head group
NP = 1024                # pooled tokens (32*32)
ND = 1024                # D-block tokens (32*32)
NS = 192                 # small-block tokens
P = 128

_CACHED = {}


def _build_nc():
    import concourse.bass as bass
    import concourse.tile as tile
    from concourse import bacc, mybir
    from concourse.masks import make_identity

    f32 = mybir.dt.float32
    bf16 = mybir.dt.bfloat16
    EXP = mybir.ActivationFunctionType.Exp

    nc = bacc.Bacc(None)

    # ---- per-core I/O ----
    xp_d = nc.dram_tensor("xp", (P, 3, NP), bf16, kind="ExternalInput")
    xd_d = nc.dram_tensor("xd", (P, 3, ND), bf16, kind="ExternalInput")
    xs_d = nc.dram_tensor("xs", (P, 3, 4 * NS), bf16, kind="ExternalInput")
    gwq_d = nc.dram_tensor("gwq", (P, 3, HG_C), bf16, kind="ExternalInput")
    gwk_d = nc.dram_tensor("gwk", (P, 3, HG_C), bf16, kind="ExternalInput")
    asq_d = nc.dram_tensor("asq", (P, 3, HG_C), bf16, kind="ExternalInput")
    ask_d = nc.dram_tensor("ask", (P, 3, HG_C), bf16, kind="ExternalInput")
    asv_d = nc.dram_tensor("asv", (P, 3, HG_C), bf16, kind="ExternalInput")
    wp_d = nc.dram_tensor("wp", (HG_C, C), bf16, kind="ExternalInput")
    bias_d = nc.dram_tensor("bias", (HG_C, 4), f32, kind="ExternalInput")
    bvrow_d = nc.dram_tensor("bvrow", (1, HG_C), bf16, kind="ExternalInput")

    attnw_d = nc.dram_tensor("attnw", (HPG, NP, NP), f32, kind="ExternalOutput")
    fpd_d = nc.dram_tensor("fpd", (P, 3, ND), f32, kind="ExternalOutput")
    fps_d = nc.dram_tensor("fps", (P, 3, 4 * NS), f32, kind="ExternalOutput")

    def mm(out, lhsT, rhs, **kw):
        nc.tensor.matmul(out, lhsT, rhs, **kw)

    with tile.TileContext(nc) as tc:
        with tc.tile_pool(name="const", bufs=1) as consts:
            ident = consts.tile([P, P], bf16)
            make_identity(nc, ident[:])
            ones_sb = consts.tile([1, P], bf16)
            nc.vector.memset(ones_sb[:], 1.0)

            xp_sb = consts.tile([P, 3, NP], bf16)
            nc.sync.dma_start(xp_sb[:], xp_d[:])
            xd_sb = consts.tile([P, 3, ND], bf16)
            nc.sync.dma_start(xd_sb[:], xd_d[:])
            xs_sb = consts.tile([P, 3, 4 * NS], bf16)
            nc.sync.dma_start(xs_sb[:], xs_d[:])
            gwq_sb = consts.tile([P, 3, HG_C], bf16)
            nc.sync.dma_start(gwq_sb[:], gwq_d[:])
            gwk_sb = consts.tile([P, 3, HG_C], bf16)
            nc.sync.dma_start(gwk_sb[:], gwk_d[:])
            asq_sb = consts.tile([P, 3, HG_C], bf16)
            nc.sync.dma_start(asq_sb[:], asq_d[:])
            ask_sb = consts.tile([P, 3, HG_C], bf16)
            nc.sync.dma_start(ask_sb[:], ask_d[:])
            asv_sb = consts.tile([P, 3, HG_C], bf16)
            nc.sync.dma_start(asv_sb[:], asv_d[:])
            wp_sb = consts.tile([HG_C, C], bf16)
            nc.sync.dma_start(wp_sb[:], wp_d[:])
            bias_sb = consts.tile([HG_C, 4], f32)
            nc.sync.dma_start(bias_sb[:], bias_d[:])
            bvrow_sb = consts.tile([1, HG_C], bf16)
            nc.sync.dma_start(bvrow_sb[:], bvrow_d[:])

            # staging tiles that persist across phases
            gw_qT = consts.tile([HG_C, NP], bf16)
            gw_kT = consts.tile([HG_C, NP], bf16)
            as_qT = consts.tile([HG_C, ND], bf16)
            as_kT = consts.tile([HG_C, ND], bf16)
            v1_sb = consts.tile([P, HPG, 8, DH + 1], bf16)   # D: [v_h | ones]
            nc.vector.memset(v1_sb[:], 1.0)
            ocn_sb = consts.tile([P, 8, HG_C], bf16)         # D: normalized out (n-part)
            outTn_sb = consts.tile([HG_C, ND], bf16)         # D: normalized out^T
            fpd_sb = consts.tile([P, 3, ND], f32)
            qTs = consts.tile([HG_C, 4 * NS], bf16)
            kTs = consts.tile([HG_C, 4 * NS], bf16)
            v1s_sb = consts.tile([HG_C, 4, 2, HPG, DH + 1], bf16)  # small v' per (blk, mchunk, h)
            nc.vector.memset(v1s_sb[:], 1.0)
            ocns_sb = consts.tile([HG_C, 4, 2, HG_C], bf16)
            outTsn_sb = consts.tile([HG_C, 4 * NS], bf16)
            fps_sb = consts.tile([P, 3, 4 * NS], f32)

            # ---------- phase 1: qkv projections (gw q/k, D q/k/v, small q/k/v) ----------
            with tc.tile_pool(name="qk_ps", bufs=2, space="PSUM") as pqk, \
                 tc.tile_pool(name="v_ps", bufs=2, space="PSUM") as pv:
                for (src, w_sb, b_col, dst, n_tok) in (
                    (xp_sb, gwq_sb, 0, gw_qT, NP),
                    (xp_sb, gwk_sb, 1, gw_kT, NP),
                    (xd_sb, asq_sb, 2, as_qT, ND),
                    (xd_sb, ask_sb, 3, as_kT, ND),
                    (xs_sb, asq_sb, 2, qTs, 4 * NS),
                    (xs_sb, ask_sb, 3, kTs, 4 * NS),
                ):
                    q_ps = pqk.tile([HG_C, n_tok], f32, tag="qk")
                    for (c0, cw) in ((0, 512), (512, n_tok - 512)):
                        for k in range(3):
                            mm(q_ps[:, c0:c0 + cw],
                               w_sb[:, k, :], src[:, k, c0:c0 + cw],
                               start=(k == 0), stop=(k == 2))
                    nc.vector.tensor_scalar_add(dst[:], q_ps[:], bias_sb[:, b_col:b_col + 1])

                # D-path v, natural layout (m on partitions), with bias via K=1 ones matmul
                for mc in range(8):
                    v_ps = pv.tile([P, HG_C], f32, tag="v")
                    for k in range(3):
                        mm(v_ps[:], xd_sb[:, k, mc * P:(mc + 1) * P], asv_sb[:, k, :],
                           start=(k == 0), stop=False)
                    mm(v_ps[:], ones_sb[:, 0:P], bvrow_sb[:], start=False, stop=True)
                    nc.vector.tensor_copy(
                        v1_sb[:, :, mc, 0:DH],
                        v_ps[:].rearrange("p (h d) -> p h d", h=HPG))

                # small-path v
                for j in range(4):
                    for mc in range(2):
                        vs_ps = pv.tile([96, HG_C], f32, tag="vs")
                        for k in range(3):
                            mm(vs_ps[:], xs_sb[:, k, j * NS + mc * 96: j * NS + (mc + 1) * 96],
                               asv_sb[:, k, :], start=(k == 0), stop=False)
                        mm(vs_ps[:], ones_sb[:, 0:96], bvrow_sb[:], start=False, stop=True)
                        nc.vector.tensor_copy(
                            v1s_sb[:96, j, mc, :, 0:DH],
                            vs_ps[:].rearrange("p (h d) -> p h d", h=HPG))


            # ---------- phases 2+3 interleaved: gw attention + D attention ----------
            # gw unit (nb, halfg): 3 strip-packed S matmuls + 3 exps (+ finish on
            # halfg==1).  D unit (half, mc): 3 strip-packed S^T matmuls + 3 exps +
            # 3 AV matmuls for the PREVIOUS mc (software pipeline, keeps PE from
            # stalling on the exp).  1:1 interleave spreads ACT/PE/DMA load.
            with tc.tile_pool(name="o_sb", bufs=1) as posb:
                outT_sb = [posb.tile([DH + 1, ND], bf16, tag=f"osb{h}", name=f"osb{h}")
                           for h in range(HPG)]
                with tc.tile_pool(name="s_ps", bufs=3, space="PSUM") as ps, \
                     tc.tile_pool(name="p_sb", bufs=3) as pp, \
                     tc.tile_pool(name="r_sb", bufs=12) as pr, \
                     tc.tile_pool(name="st_ps", bufs=2, space="PSUM") as pst, \
                     tc.tile_pool(name="ot_ps", bufs=1, space="PSUM") as pot, \
                     tc.tile_pool(name="pt_sb", bufs=6) as ppt:
                    gw_p = {}   # nb -> p_sb tile
                    gw_r = {}   # (nb, h, halfg) -> r part
                    def gw_unit(u):
                        nb, halfg = divmod(u, 2)
                        if halfg == 0:
                            gw_p[nb] = [pp.tile([P, NP], f32, tag="p", name=f"p{nb}_{h}")
                                        for h in range(HPG)]
                        s_tiles = []
                        for h in range(HPG):
                            hs = h * DH
                            s_ps = ps.tile([P, 512], f32, tag="s", name=f"s{u}_{h}")
                            mm(s_ps[:], gw_qT[hs:hs + DH, nb * P:(nb + 1) * P],
                               gw_kT[hs:hs + DH, halfg * 512:(halfg + 1) * 512],
                               start=True, stop=True, tile_position=(hs, 0))
                            s_tiles.append(s_ps)
                        for h in range(HPG):
                            r = pr.tile([P, 1], f32, tag="r", name=f"r{u}_{h}")
                            nc.scalar.activation(
                                gw_p[nb][h][:, halfg * 512:(halfg + 1) * 512],
                                s_tiles[h][:], EXP, accum_out=r[:])
                            gw_r[(nb, h, halfg)] = r
                        if halfg == 1:
                            for h in range(HPG):
                                rt = pr.tile([P, 1], f32, tag="rt", name=f"rt{u}_{h}")
                                nc.vector.tensor_tensor(
                                    rt[:], gw_r[(nb, h, 0)][:], gw_r[(nb, h, 1)][:],
                                    mybir.AluOpType.add)
                                rinv = pr.tile([P, 1], f32, tag="ri", name=f"ri{u}_{h}")
                                nc.vector.reciprocal(rinv[:], rt[:])
                                nc.vector.tensor_scalar_mul(
                                    gw_p[nb][h][:], gw_p[nb][h][:], rinv[:])
                                nc.sync.dma_start(
                                    attnw_d[h, nb * P:(nb + 1) * P, :], gw_p[nb][h][:])

                    for half in range(2):
                        outT = [pot.tile([DH + 1, 512], f32, tag=f"ot{h}", name=f"ot{h}")
                                for h in range(HPG)]
                        prev_pt = None
                        for mc in range(8):
                            cur_pt = []
                            for h in range(HPG):
                                hs = h * DH
                                st_ps = pst.tile([P, 512], f32, tag="st")
                                mm(st_ps[:], as_kT[hs:hs + DH, mc * P:(mc + 1) * P],
                                   as_qT[hs:hs + DH, half * 512:(half + 1) * 512],
                                   start=True, stop=True, tile_position=(hs, 0))
                                pt_sb = ppt.tile([P, 512], bf16, tag="pt",
                                                 name=f"pt{half}_{mc}_{h}")
                                nc.scalar.activation(pt_sb[:], st_ps[:], EXP)
                                cur_pt.append(pt_sb)
                            if prev_pt is not None:
                                for h in range(HPG):
                                    mm(outT[h][:], v1_sb[:, h, mc - 1, :], prev_pt[h][:],
                                       start=(mc - 1 == 0), stop=False)
                            prev_pt = cur_pt
                            gw_unit(half * 8 + mc)
                        for h in range(HPG):
                            mm(outT[h][:], v1_sb[:, h, 7, :], prev_pt[h][:],
                               start=False, stop=True)
                        for h in range(HPG):
                            nc.vector.tensor_copy(
                                outT_sb[h][:, half * 512:(half + 1) * 512], outT[h][:])

                # transpose per (chunk, head), normalize by r (= row DH), collect n-part
                with tc.tile_pool(name="t_ps", bufs=4, space="PSUM") as ptr, \
                     tc.tile_pool(name="tb_ps", bufs=2, space="PSUM") as ptb, \
                     tc.tile_pool(name="ri2", bufs=8) as pri:
                    for cb in range(8):
                        for h in range(HPG):
                            t_ps = ptr.tile([P, DH + 1], bf16, tag="t")
                            nc.tensor.transpose(
                                t_ps[:], outT_sb[h][:, cb * P:(cb + 1) * P],
                                ident[0:DH + 1, 0:DH + 1])
                            rinv = pri.tile([P, 1], f32, tag="ri2")
                            nc.vector.reciprocal(rinv[:], t_ps[:, DH:DH + 1])
                            nc.vector.tensor_scalar_mul(
                                ocn_sb[:, cb, h * DH:(h + 1) * DH], t_ps[:, 0:DH], rinv[:])
                    # transpose back to (c_in, n)
                    for cb in range(8):
                        tb_ps = ptb.tile([HG_C, P], bf16, tag="tb")
                        nc.tensor.transpose(tb_ps[:], ocn_sb[:, cb, :], ident[:])
                        nc.vector.tensor_copy(outTn_sb[:, cb * P:(cb + 1) * P], tb_ps[:])

            # partial projection: fpd = wp[hg rows]^T @ outTn
            with tc.tile_pool(name="f_ps", bufs=2, space="PSUM") as pf:
                for co in range(3):
                    f_ps = pf.tile([P, ND], f32, tag="f")
                    for half in range(2):
                        mm(f_ps[:, half * 512:(half + 1) * 512],
                           wp_sb[:, co * P:(co + 1) * P],
                           outTn_sb[:, half * 512:(half + 1) * 512],
                           start=True, stop=True)
                    nc.vector.tensor_copy(fpd_sb[:, co, :], f_ps[:])
            nc.sync.dma_start(fpd_d[:], fpd_sb[:])

            # ---------- phase 4: small blocks ----------
            with tc.tile_pool(name="os_sb", bufs=12) as possb:
                outTs_sb = {}
                with tc.tile_pool(name="sts_ps", bufs=4, space="PSUM") as psts, \
                     tc.tile_pool(name="ots_ps", bufs=3, space="PSUM") as pots, \
                     tc.tile_pool(name="pts_sb", bufs=6) as ppts:
                    for j in range(4):
                        for h in range(HPG):
                            hs = h * DH
                            st_ps = psts.tile([96, 2, NS], f32, tag="sts")
                            for mc in range(2):
                                mm(st_ps[:, mc, :],
                                   kTs[hs:hs + DH, j * NS + mc * 96: j * NS + (mc + 1) * 96],
                                   qTs[hs:hs + DH, j * NS:(j + 1) * NS],
                                   start=True, stop=True, tile_position=(hs, 0))
                            pt_sb = ppts.tile([96, 2, NS], bf16, tag="pts")
                            nc.scalar.activation(pt_sb[:], st_ps[:], EXP)
                            ot_ps = pots.tile([DH + 1, NS], f32, tag="ots")
                            for mc in range(2):
                                mm(ot_ps[:], v1s_sb[:96, j, mc, h, :], pt_sb[:, mc, :],
                                   start=(mc == 0), stop=(mc == 1))
                            o_sb = possb.tile([DH + 1, NS], bf16, tag="oss")
                            nc.vector.tensor_copy(o_sb[:], ot_ps[:])
                            outTs_sb[(j, h)] = o_sb
                with tc.tile_pool(name="ts_ps", bufs=4, space="PSUM") as ptrs, \
                     tc.tile_pool(name="tbs_ps", bufs=2, space="PSUM") as ptbs, \
                     tc.tile_pool(name="ri3", bufs=8) as pri3:
                    for j in range(4):
                        for cb in range(2):
                            for h in range(HPG):
                                t_ps = ptrs.tile([96, DH + 1], bf16, tag="ts")
                                nc.tensor.transpose(
                                    t_ps[:], outTs_sb[(j, h)][:, cb * 96:(cb + 1) * 96],
                                    ident[0:DH + 1, 0:DH + 1])
                                rinv = pri3.tile([96, 1], f32, tag="ri3")
                                nc.vector.reciprocal(rinv[:], t_ps[:, DH:DH + 1])
                                nc.vector.tensor_scalar_mul(
                                    ocns_sb[:96, j, cb, h * DH:(h + 1) * DH],
                                    t_ps[:, 0:DH], rinv[:])
                        for cb in range(2):
                            tb_ps = ptbs.tile([HG_C, 96], bf16, tag="tbs")
                            nc.tensor.transpose(tb_ps[:], ocns_sb[:96, j, cb, :],
                                                ident[0:96, 0:96])
                            nc.vector.tensor_copy(
                                outTsn_sb[:, j * NS + cb * 96: j * NS + (cb + 1) * 96],
                                tb_ps[:])
            with tc.tile_pool(name="fs_ps", bufs=2, space="PSUM") as pfs:
                for co in range(3):
                    f_ps = pfs.tile([P, 4 * NS], f32, tag="fs")
                    for (c0, cw) in ((0, 512), (512, 256)):
                        mm(f_ps[:, c0:c0 + cw],
                           wp_sb[:, co * P:(co + 1) * P],
                           outTsn_sb[:, c0:c0 + cw],
                           start=True, stop=True)
                    nc.vector.tensor_copy(fps_sb[:, co, :], f_ps[:])
            nc.sync.dma_start(fps_d[:], fps_sb[:])

    nc.compile()
    return nc


# ---------------- host-side numpy helpers ----------------

def _avgpool2(x):
    b, c, h, w = x.shape
    return x.reshape(b, c, h // 2, 2, w // 2, 2).mean(axis=(3, 5))


def _up2_align_corners(x):
    # x: (C, H, W) -> (C, 2H, 2W), bilinear align_corners=True
    c, h, w = x.shape
    hn, wn = 2 * h, 2 * w

    def coords(n_out, n_in):
        s = np.arange(n_out) * ((n_in - 1) / (n_out - 1))
        i0 = np.floor(s).astype(np.int32)
        i1 = np.minimum(i0 + 1, n_in - 1)
        wt = (s - i0).astype(np.float32)
        return i0, i1, wt

    y0, y1, wy = coords(hn, h)
    x0, x1, wx = coords(wn, w)
    row = x[:, y0, :] * (1.0 - wy)[None, :, None] + x[:, y1, :] * wy[None, :, None]
    return row[:, :, x0] * (1.0 - wx) + row[:, :, x1] * wx


def _zscore(y):
    # y: (C, H, W); per-column stats over C*H (population std)
    c, h, w = y.shape
    f = y.reshape(c * h, w)
    mu = f.mean(axis=0)
    sd = f.std(axis=0)
    sd = np.where(sd == 0, 1.0, sd).astype(np.float32)
    return ((f - mu) / sd).reshape(c, h, w).astype(np.float32)


def _bf16(a):
    import ml_dtypes
    return np.ascontiguousarray(a).astype(ml_dtypes.bfloat16)


def _chunk3(a):
    # (384, n) -> (128, 3, n) with row = ko*128 + p, cast to bf16
    n = a.shape[1]
    return _bf16(a.reshape(3, P, n).transpose(1, 0, 2))


def _prep_inputs(x, gw_Wqkv, gw_bqkv, as_Wqkv, as_bqkv, as_Wproj):
    x = np.asarray(x, dtype=np.float32)
    xc = np.ascontiguousarray(x.transpose(0, 3, 1, 2))           # (B, C, 64, 64)
    x0 = _avgpool2(xc)                                           # (B, C, 32, 32)
    xpT = x0.reshape(B, C, NP)                                   # == xp^T per batch
    xdT = np.ascontiguousarray(xc[:, :, 16:48, 16:48]).reshape(B, C, ND)
    blocks = [x0[:, :, 0:8, 0:24], x0[:, :, 24:32, 8:32],
              x0[:, :, 8:32, 0:8], x0[:, :, 0:24, 24:32]]
    xsT = np.concatenate(
        [np.ascontiguousarray(blk).reshape(B, C, NS) for blk in blocks], axis=2)

    gw_Wqkv = np.asarray(gw_Wqkv, np.float32)
    as_Wqkv = np.asarray(as_Wqkv, np.float32)
    gw_bqkv = np.asarray(gw_bqkv, np.float32)
    as_bqkv = np.asarray(as_bqkv, np.float32)
    as_Wproj = np.asarray(as_Wproj, np.float32)

    in_maps = []
    for core in range(8):
        b, hg = divmod(core, 4)
        cs = slice(hg * HG_C, (hg + 1) * HG_C)
        csa = np.arange(hg * HG_C, (hg + 1) * HG_C)
        m = {
            "xp": _chunk3(xpT[b]),
            "xd": _chunk3(xdT[b]),
            "xs": _chunk3(xsT[b]),
            "gwq": _chunk3(gw_Wqkv[:, csa] * SCALE),
            "gwk": _chunk3(gw_Wqkv[:, C + csa]),
            "asq": _chunk3(as_Wqkv[:, csa] * SCALE),
            "ask": _chunk3(as_Wqkv[:, C + csa]),
            "asv": _chunk3(as_Wqkv[:, 2 * C + csa]),
            "wp": _bf16(as_Wproj[cs, :]),
            "bias": np.ascontiguousarray(np.stack(
                [gw_bqkv[csa] * SCALE, gw_bqkv[C + csa],
                 as_bqkv[csa] * SCALE, as_bqkv[C + csa]], axis=1)),
            "bvrow": _bf16(as_bqkv[2 * C + csa][None, :]),
        }
        in_maps.append(m)
    return in_maps


def _postprocess(results, as_bproj):
    bp = np.asarray(as_bproj, np.float32)
    attn_weight = np.empty((B, NH, NP, NP), np.float32)
    attn_all = np.zeros((B, C, 64, 64), np.float32)
    for b in range(B):
        fpd = np.zeros((C, ND), np.float32)
        fps = np.zeros((C, 4 * NS), np.float32)
        for hg in range(NG):
            r = results[b * 4 + hg]
            attn_weight[b, hg * HPG:(hg + 1) * HPG] = r["attnw"]
            fpd += r["fpd"].transpose(1, 0, 2).reshape(C, ND)
            fps += r["fps"].transpose(1, 0, 2).reshape(C, 4 * NS)
        fpd += bp[:, None]
        fps += bp[:, None]
        aD = _zscore(fpd.reshape(C, 32, 32))
        small = []
        shapes = [(8, 24), (8, 24), (24, 8), (24, 8)]
        for j, (hh, ww) in enumerate(shapes):
            blk = fps[:, j * NS:(j + 1) * NS].reshape(C, hh, ww)
            small.append(_zscore(_up2_align_corners(blk)))
        aB1, aB2, aC1, aC2 = small
        attn_all[b, :, 0:16, 0:48] = aB1
        attn_all[b, :, 48:64, 16:64] = aB2
        attn_all[b, :, 16:64, 0:16] = aC1
        attn_all[b, :, 0:48, 48:64] = aC2
        attn_all[b, :, 16:48, 16:48] = aD
    return np.ascontiguousarray(attn_all.transpose(0, 2, 3, 1)), attn_weight


def _run(in_maps, trace=False, **kw):
    from concourse.bass_utils import run_bass_kernel_spmd
    if "nc" not in _CACHED:
        _CACHED["nc"] = _build_nc()
    return run_bass_kernel_spmd(_CACHED["nc"], in_maps, list(range(8)),
                                trace=trace, **kw)


def kernel(x, Cam_Centre, gw_Wqkv, gw_bqkv, as_Wqkv, as_bqkv, as_Wproj, as_bproj):
    in_maps = _prep_inputs(x, gw_Wqkv, gw_bqkv, as_Wqkv, as_bqkv, as_Wproj)
    res = _run(in_maps)
    return _postprocess(res.results, as_bproj)


# revision 20
# speedup vs baseline: 3.0908x; 1.0037x over previous
"""Trainium2 Bass kernel for nn_Eye_Center (sparse_attention).

Sharding: 8 cores = (batch B=2) x (4 head-groups of 3 heads each).
Each core computes, for its (b, head-group):
  - gw path:  attn_weight[b, 3hg:3hg+3] = softmax(q k^T) on the pooled image
  - D path:   3 heads of full attention (N=1024) + partial projection
  - 4 small blocks (N=192): 3 heads of attention + partial projection
Host does: avgpool / slicing (pre), partial-proj sum over head groups,
bias add, bilinear upsample, zscore, scatter (post). All O(MB) numpy work.
"""

import numpy as np

# ---- problem constants (hardcoded per the task contract) ----
B = 2
C = 384
NH = 12
DH = 32
SCALE = DH ** -0.5
HPG = 3                  # heads per group
NG = 4                   # head groups
HG_C = HPG * DH          # 96 channels per head group
NP = 1024                # pooled tokens (32*32)
ND = 1024                # D-block tokens (32*32)
NS = 192                 # small-block tokens
P = 128

_CACHED = {}


def _build_nc():
    import concourse.bass as bass
    import concourse.tile as tile
    from concourse import bacc, mybir
    from concourse.masks import make_identity

    f32 = mybir.dt.float32
    bf16 = mybir.dt.bfloat16
    EXP = mybir.ActivationFunctionType.Exp

    nc = bacc.Bacc(None)

    # ---- per-core I/O ----
    xp_d = nc.dram_tensor("xp", (P, 3, NP), bf16, kind="ExternalInput")
    xd_d = nc.dram_tensor("xd", (P, 3, ND), bf16, kind="ExternalInput")
    xs_d = nc.dram_tensor("xs", (P, 3, 4 * NS), bf16, kind="ExternalInput")
    gwq_d = nc.dram_tensor("gwq", (P, 3, HG_C), bf16, kind="ExternalInput")
    gwk_d = nc.dram_tensor("gwk", (P, 3, HG_C), bf16, kind="ExternalInput")
    asq_d = nc.dram_tensor("asq", (P, 3, HG_C), bf16, kind="ExternalInput")
    ask_d = nc.dram_tensor("ask", (P, 3, HG_C), bf16, kind="ExternalInput")
    asv_d = nc.dram_tensor("asv", (P, 3, HG_C), bf16, kind="ExternalInput")
    wp_d = nc.dram_tensor("wp", (HG_C, C), bf16, kind="ExternalInput")
    bias_d = nc.dram_tensor("bias", (HG_C, 4), f32, kind="ExternalInput")
    bvrow_d = nc.dram_tensor("bvrow", (1, HG_C), bf16, kind="ExternalInput")

    attnw_d = nc.dram_tensor("attnw", (HPG, NP, NP), f32, kind="ExternalOutput")
    fpd_d = nc.dram_tensor("fpd", (P, 3, ND), f32, kind="ExternalOutput")
    fps_d = nc.dram_tensor("fps", (P, 3, 4 * NS), f32, kind="ExternalOutput")

    def mm(out, lhsT, rhs, **kw):
        nc.tensor.matmul(out, lhsT, rhs, **kw)

    with tile.TileContext(nc) as tc:
        with tc.tile_pool(name="const", bufs=1) as consts:
            ident = consts.tile([P, P], bf16)
            make_identity(nc, ident[:])
            ones_sb = consts.tile([1, P], bf16)
            nc.vector.memset(ones_sb[:], 1.0)

            xp_sb = consts.tile([P, 3, NP], bf16)
            nc.sync.dma_start(xp_sb[:], xp_d[:])
            xd_sb = consts.tile([P, 3, ND], bf16)
            nc.sync.dma_start(xd_sb[:], xd_d[:])
            xs_sb = consts.tile([P, 3, 4 * NS], bf16)
            nc.sync.dma_start(xs_sb[:], xs_d[:])
            gwq_sb = consts.tile([P, 3, HG_C], bf16)
            nc.sync.dma_start(gwq_sb[:], gwq_d[:])
            gwk_sb = consts.tile([P, 3, HG_C], bf16)
            nc.sync.dma_start(gwk_sb[:], gwk_d[:])
            asq_sb = consts.tile([P, 3, HG_C], bf16)
            nc.sync.dma_start(asq_sb[:], asq_d[:])
            ask_sb = consts.tile([P, 3, HG_C], bf16)
            nc.sync.dma_start(ask_sb[:], ask_d[:])
            asv_sb = consts.tile([P, 3, HG_C], bf16)
            nc.sync.dma_start(asv_sb[:], asv_d[:])
            wp_sb = consts.tile([HG_C, C], bf16)
            nc.sync.dma_start(wp_sb[:], wp_d[:])
            bias_sb = consts.tile([HG_C, 4], f32)
            nc.sync.dma_start(bias_sb[:], bias_d[:])
            bvrow_sb = consts.tile([1, HG_C], bf16)
            nc.sync.dma_start(bvrow_sb[:], bvrow_d[:])

            # staging tiles that persist across phases
            gw_qT = consts.tile([HG_C, NP], bf16)
            gw_kT = consts.tile([HG_C, NP], bf16)
            as_qT = consts.tile([HG_C, ND], bf16)
            as_kT = consts.tile([HG_C, ND], bf16)
            v1_sb = consts.tile([P, HPG, 8, DH + 1], bf16)   # D: [v_h | ones]
            nc.vector.memset(v1_sb[:], 1.0)
            ocn_sb = consts.tile([P, 8, HG_C], bf16)         # D: normalized out (n-part)
            outTn_sb = consts.tile([HG_C, ND], bf16)         # D: normalized out^T
            fpd_sb = consts.tile([P, 3, ND], f32)
            qTs = consts.tile([HG_C, 4 * NS], bf16)
            kTs = consts.tile([HG_C, 4 * NS], bf16)
            v1s_sb = consts.tile([HG_C, 4, 2, HPG, DH + 1], bf16)  # small v' per (blk, mchunk, h)
            nc.vector.memset(v1s_sb[:], 1.0)
            ocns_sb = consts.tile([HG_C, 4, 2, HG_C], bf16)
            outTsn_sb = consts.tile([HG_C, 4 * NS], bf16)
            fps_sb = consts.tile([P, 3, 4 * NS], f32)

            # ---------- phase 1: qkv projections (gw q/k, D q/k/v, small q/k/v) ----------
            with tc.tile_pool(name="qk_ps", bufs=2, space="PSUM") as pqk, \
                 tc.tile_pool(name="v_ps", bufs=2, space="PSUM") as pv:
                for (src, w_sb, b_col, dst, n_tok) in (
                    (xp_sb, gwq_sb, 0, gw_qT, NP),
                    (xp_sb, gwk_sb, 1, gw_kT, NP),
                    (xd_sb, asq_sb, 2, as_qT, ND),
                    (xd_sb, ask_sb, 3, as_kT, ND),
                    (xs_sb, asq_sb, 2, qTs, 4 * NS),
                    (xs_sb, ask_sb, 3, kTs, 4 * NS),
                ):
                    q_ps = pqk.tile([HG_C, n_tok], f32, tag="qk")
                    for (c0, cw) in ((0, 512), (512, n_tok - 512)):
                        for k in range(3):
                            mm(q_ps[:, c0:c0 + cw],
                               w_sb[:, k, :], src[:, k, c0:c0 + cw],
                               start=(k == 0), stop=(k == 2))
                    nc.vector.tensor_scalar_add(dst[:], q_ps[:], bias_sb[:, b_col:b_col + 1])

                # D-path v, natural layout (m on partitions), with bias via K=1 ones matmul
                for mc in range(8):
                    v_ps = pv.tile([P, HG_C], f32, tag="v")
                    for k in range(3):
                        mm(v_ps[:], xd_sb[:, k, mc * P:(mc + 1) * P], asv_sb[:, k, :],
                           start=(k == 0), stop=False)
                    mm(v_ps[:], ones_sb[:, 0:P], bvrow_sb[:], start=False, stop=True)
                    nc.vector.tensor_copy(
                        v1_sb[:, :, mc, 0:DH],
                        v_ps[:].rearrange("p (h d) -> p h d", h=HPG))

                # small-path v
                for j in range(4):
                    for mc in range(2):
                        vs_ps = pv.tile([96, HG_C], f32, tag="vs")
                        for k in range(3):
                            mm(vs_ps[:], xs_sb[:, k, j * NS + mc * 96: j * NS + (mc + 1) * 96],
                               asv_sb[:, k, :], start=(k == 0), stop=False)
                        mm(vs_ps[:], ones_sb[:, 0:96], bvrow_sb[:], start=False, stop=True)
                        nc.vector.tensor_copy(
                            v1s_sb[:96, j, mc, :, 0:DH],
                            vs_ps[:].rearrange("p (h d) -> p h d", h=HPG))


            # ---------- phases 2+3 interleaved: gw attention + D attention ----------
            # gw unit (nb, halfg): 3 strip-packed S matmuls + 3 exps (+ finish on
            # halfg==1).  D unit (half, mc): 3 strip-packed S^T matmuls + 3 exps +
            # 3 AV matmuls for the PREVIOUS mc (software pipeline, keeps PE from
            # stalling on the exp).  1:1 interleave spreads ACT/PE/DMA load.
            with tc.tile_pool(name="o_sb", bufs=1) as posb:
                outT_sb = [posb.tile([DH + 1, ND], bf16, tag=f"osb{h}", name=f"osb{h}")
                           for h in range(HPG)]
                with tc.tile_pool(name="s_ps", bufs=3, space="PSUM") as ps, \
                     tc.tile_pool(name="p_sb", bufs=3) as pp, \
                     tc.tile_pool(name="r_sb", bufs=12) as pr, \
                     tc.tile_pool(name="st_ps", bufs=2, space="PSUM") as pst, \
                     tc.tile_pool(name="ot_ps", bufs=1, space="PSUM") as pot, \
                     tc.tile_pool(name="pt_sb", bufs=26) as ppt:
                    gw_p = {}   # nb -> p_sb tile
                    gw_r = {}   # (nb, h, halfg) -> r part
                    def gw_unit(u):
                        nb, halfg = divmod(u, 2)
                        if halfg == 0:
                            gw_p[nb] = [pp.tile([P, NP], f32, tag="p", name=f"p{nb}_{h}")
                                        for h in range(HPG)]
                        s_tiles = []
                        for h in range(HPG):
                            hs = h * DH
                            s_ps = ps.tile([P, 512], f32, tag="s", name=f"s{u}_{h}")
                            mm(s_ps[:], gw_qT[hs:hs + DH, nb * P:(nb + 1) * P],
                               gw_kT[hs:hs + DH, halfg * 512:(halfg + 1) * 512],
                               start=True, stop=True, tile_position=(hs, 0))
                            s_tiles.append(s_ps)
                        for h in range(HPG):
                            r = pr.tile([P, 1], f32, tag="r", name=f"r{u}_{h}")
                            nc.scalar.activation(
                                gw_p[nb][h][:, halfg * 512:(halfg + 1) * 512],
                                s_tiles[h][:], EXP, accum_out=r[:])
                            gw_r[(nb, h, halfg)] = r
                        if halfg == 1:
                            for h in range(HPG):
                                rt = pr.tile([P, 1], f32, tag="rt", name=f"rt{u}_{h}")
                                nc.vector.tensor_tensor(
                                    rt[:], gw_r[(nb, h, 0)][:], gw_r[(nb, h, 1)][:],
                                    mybir.AluOpType.add)
                                rinv = pr.tile([P, 1], f32, tag="ri", name=f"ri{u}_{h}")
                                nc.vector.reciprocal(rinv[:], rt[:])
                                nc.vector.tensor_scalar_mul(
                                    gw_p[nb][h][:], gw_p[nb][h][:], rinv[:])
                                nc.sync.dma_start(
                                    attnw_d[h, nb * P:(nb + 1) * P, :], gw_p[nb][h][:])

                    for half in range(2):
                        outT = [pot.tile([DH + 1, 512], f32, tag=f"ot{h}", name=f"ot{h}")
                                for h in range(HPG)]
                        pts = {}
                        for mc in range(8):
                            for h in range(HPG):
                                hs = h * DH
                                st_ps = pst.tile([P, 512], f32, tag="st")
                                mm(st_ps[:], as_kT[hs:hs + DH, mc * P:(mc + 1) * P],
                                   as_qT[hs:hs + DH, half * 512:(half + 1) * 512],
                                   start=True, stop=True, tile_position=(hs, 0))
                                pt_sb = ppt.tile([P, 512], bf16, tag="pt",
                                                 name=f"pt{half}_{mc}_{h}")
                                nc.scalar.activation(pt_sb[:], st_ps[:], EXP)
                                pts[(mc, h)] = pt_sb
                            gw_unit(half * 8 + mc)
                        # all AV matmuls after the S groups: their exps are long done,
                        # so the PE stream stays dense and strip-packing unbroken
                        for mc in range(8):
                            for h in range(HPG):
                                mm(outT[h][:], v1_sb[:, h, mc, :], pts[(mc, h)][:],
                                   start=(mc == 0), stop=(mc == 7))
                        for h in range(HPG):
                            nc.vector.tensor_copy(
                                outT_sb[h][:, half * 512:(half + 1) * 512], outT[h][:])

                # transpose per (chunk, head), normalize by r (= row DH), collect n-part
                with tc.tile_pool(name="t_ps", bufs=4, space="PSUM") as ptr, \
                     tc.tile_pool(name="tb_ps", bufs=2, space="PSUM") as ptb, \
                     tc.tile_pool(name="ri2", bufs=8) as pri:
                    for cb in range(8):
                        for h in range(HPG):
                            t_ps = ptr.tile([P, DH + 1], bf16, tag="t")
                            nc.tensor.transpose(
                                t_ps[:], outT_sb[h][:, cb * P:(cb + 1) * P],
                                ident[0:DH + 1, 0:DH + 1])
                            rinv = pri.tile([P, 1], f32, tag="ri2")
                            nc.vector.reciprocal(rinv[:], t_ps[:, DH:DH + 1])
                            nc.vector.tensor_scalar_mul(
                                ocn_sb[:, cb, h * DH:(h + 1) * DH], t_ps[:, 0:DH], rinv[:])
                    # transpose back to (c_in, n)
                    for cb in range(8):
                        tb_ps = ptb.tile([HG_C, P], bf16, tag="tb")
                        nc.tensor.transpose(tb_ps[:], ocn_sb[:, cb, :], ident[:])
                        nc.vector.tensor_copy(outTn_sb[:, cb * P:(cb + 1) * P], tb_ps[:])

            # partial projection: fpd = wp[hg rows]^T @ outTn
            with tc.tile_pool(name="f_ps", bufs=2, space="PSUM") as pf:
                for co in range(3):
                    f_ps = pf.tile([P, ND], f32, tag="f")
                    for half in range(2):
                        mm(f_ps[:, half * 512:(half + 1) * 512],
                           wp_sb[:, co * P:(co + 1) * P],
                           outTn_sb[:, half * 512:(half + 1) * 512],
                           start=True, stop=True)
                    nc.vector.tensor_copy(fpd_sb[:, co, :], f_ps[:])
            nc.sync.dma_start(fpd_d[:], fpd_sb[:])

            # ---------- phase 4: small blocks ----------
            with tc.tile_pool(name="os_sb", bufs=12) as possb:
                outTs_sb = {}
                with tc.tile_pool(name="sts_ps", bufs=4, space="PSUM") as psts, \
                     tc.tile_pool(name="ots_ps", bufs=3, space="PSUM") as pots, \
                     tc.tile_pool(name="pts_sb", bufs=6) as ppts:
                    for j in range(4):
                        for h in range(HPG):
                            hs = h * DH
                            st_ps = psts.tile([96, 2, NS], f32, tag="sts")
                            for mc in range(2):
                                mm(st_ps[:, mc, :],
                                   kTs[hs:hs + DH, j * NS + mc * 96: j * NS + (mc + 1) * 96],
                                   qTs[hs:hs + DH, j * NS:(j + 1) * NS],
                                   start=True, stop=True, tile_position=(hs, 0))
                            pt_sb = ppts.tile([96, 2, NS], bf16, tag="pts")
                            nc.scalar.activation(pt_sb[:], st_ps[:], EXP)
                            ot_ps = pots.tile([DH + 1, NS], f32, tag="ots")
                            for mc in range(2):
                                mm(ot_ps[:], v1s_sb[:96, j, mc, h, :], pt_sb[:, mc, :],
                                   start=(mc == 0), stop=(mc == 1))
                            o_sb = possb.tile([DH + 1, NS], bf16, tag="oss")
                            nc.vector.tensor_copy(o_sb[:], ot_ps[:])
                            outTs_sb[(j, h)] = o_sb
                with tc.tile_pool(name="ts_ps", bufs=4, space="PSUM") as ptrs, \
                     tc.tile_pool(name="tbs_ps", bufs=2, space="PSUM") as ptbs, \
                     tc.tile_pool(name="ri3", bufs=8) as pri3:
                    for j in range(4):
                        for cb in range(2):
                            for h in range(HPG):
                                t_ps = ptrs.tile([96, DH + 1], bf16, tag="ts")
                                nc.tensor.transpose(
                                    t_ps[:], outTs_sb[(j, h)][:, cb * 96:(cb + 1) * 96],
                                    ident[0:DH + 1, 0:DH + 1])
                                rinv = pri3.tile([96, 1], f32, tag="ri3")
                                nc.vector.reciprocal(rinv[:], t_ps[:, DH:DH + 1])
                                nc.vector.tensor_scalar_mul(
                                    ocns_sb[:96, j, cb, h * DH:(h + 1) * DH],
                                    t_ps[:, 0:DH], rinv[:])
                        for cb in range(2):
                            tb_ps = ptbs.tile([HG_C, 96], bf16, tag="tbs")
                            nc.tensor.transpose(tb_ps[:], ocns_sb[:96, j, cb, :],
                                                ident[0:96, 0:96])
                            nc.vector.tensor_copy(
                                outTsn_sb[:, j * NS + cb * 96: j * NS + (cb + 1) * 96],
                                tb_ps[:])
            with tc.tile_pool(name="fs_ps", bufs=2, space="PSUM") as pfs:
                for co in range(3):
                    f_ps = pfs.tile([P, 4 * NS], f32, tag="fs")
                    for (c0, cw) in ((0, 512), (512, 256)):
                        mm(f_ps[:, c0:c0 + cw],
                           wp_sb[:, co * P:(co + 1) * P],
                           outTsn_sb[:, c0:c0 + cw],
                           start=True, stop=True)
                    nc.vector.tensor_copy(fps_sb[:, co, :], f_ps[:])
            nc.sync.dma_start(fps_d[:], fps_sb[:])

    nc.compile()
    return nc


# ---------------- host-side numpy helpers ----------------

def _avgpool2(x):
    b, c, h, w = x.shape
    return x.reshape(b, c, h // 2, 2, w // 2, 2).mean(axis=(3, 5))


def _up2_align_corners(x):
    # x: (C, H, W) -> (C, 2H, 2W), bilinear align_corners=True
    c, h, w = x.shape
    hn, wn = 2 * h, 2 * w

    def coords(n_out, n_in):
        s = np.arange(n_out) * ((n_in - 1) / (n_out - 1))
        i0 = np.floor(s).astype(np.int32)
        i1 = np.minimum(i0 + 1, n_in - 1)
        wt = (s - i0).astype(np.float32)
        return i0, i1, wt

    y0, y1, wy = coords(hn, h)
    x0, x1, wx = coords(wn, w)
    row = x[:, y0, :] * (1.0 - wy)[None, :, None] + x[:, y1, :] * wy[None, :, None]
    return row[:, :, x0] * (1.0 - wx) + row[:, :, x1] * wx


def _zscore(y):
    # y: (C, H, W); per-column stats over C*H (population std)
    c, h, w = y.shape
    f = y.reshape(c * h, w)
    mu = f.mean(axis=0)
    sd = f.std(axis=0)
    sd = np.where(sd == 0, 1.0, sd).astype(np.float32)
    return ((f - mu) / sd).reshape(c, h, w).astype(np.float32)


def _bf16(a):
    import ml_dtypes
    return np.ascontiguousarray(a).astype(ml_dtypes.bfloat16)


def _chunk3(a):
    # (384, n) -> (128, 3, n) with row = ko*128 + p, cast to bf16
    n = a.shape[1]
    return _bf16(a.reshape(3, P, n).transpose(1, 0, 2))


def _prep_inputs(x, gw_Wqkv, gw_bqkv, as_Wqkv, as_bqkv, as_Wproj):
    x = np.asarray(x, dtype=np.float32)
    xc = np.ascontiguousarray(x.transpose(0, 3, 1, 2))           # (B, C, 64, 64)
    x0 = _avgpool2(xc)                                           # (B, C, 32, 32)
    xpT = x0.reshape(B, C, NP)                                   # == xp^T per batch
    xdT = np.ascontiguousarray(xc[:, :, 16:48, 16:48]).reshape(B, C, ND)
    blocks = [x0[:, :, 0:8, 0:24], x0[:, :, 24:32, 8:32],
              x0[:, :, 8:32, 0:8], x0[:, :, 0:24, 24:32]]
    xsT = np.concatenate(
        [np.ascontiguousarray(blk).reshape(B, C, NS) for blk in blocks], axis=2)

    gw_Wqkv = np.asarray(gw_Wqkv, np.float32)
    as_Wqkv = np.asarray(as_Wqkv, np.float32)
    gw_bqkv = np.asarray(gw_bqkv, np.float32)
    as_bqkv = np.asarray(as_bqkv, np.float32)
    as_Wproj = np.asarray(as_Wproj, np.float32)

    in_maps = []
    for core in range(8):
        b, hg = divmod(core, 4)
        cs = slice(hg * HG_C, (hg + 1) * HG_C)
        csa = np.arange(hg * HG_C, (hg + 1) * HG_C)
        m = {
            "xp": _chunk3(xpT[b]),
            "xd": _chunk3(xdT[b]),
            "xs": _chunk3(xsT[b]),
            "gwq": _chunk3(gw_Wqkv[:, csa] * SCALE),
            "gwk": _chunk3(gw_Wqkv[:, C + csa]),
            "asq": _chunk3(as_Wqkv[:, csa] * SCALE),
            "ask": _chunk3(as_Wqkv[:, C + csa]),
            "asv": _chunk3(as_Wqkv[:, 2 * C + csa]),
            "wp": _bf16(as_Wproj[cs, :]),
            "bias": np.ascontiguousarray(np.stack(
                [gw_bqkv[csa] * SCALE, gw_bqkv[C + csa],
                 as_bqkv[csa] * SCALE, as_bqkv[C + csa]], axis=1)),
            "bvrow": _bf16(as_bqkv[2 * C + csa][None, :]),
        }
        in_maps.append(m)
    return in_maps


def _postprocess(results, as_bproj):
    bp = np.asarray(as_bproj, np.float32)
    attn_weight = np.empty((B, NH, NP, NP), np.float32)
    attn_all = np.zeros((B, C, 64, 64), np.float32)
    for b in range(B):
        fpd = np.zeros((C, ND), np.float32)
        fps = np.zeros((C, 4 * NS), np.float32)
        for hg in range(NG):
            r = results[b * 4 + hg]
            attn_weight[b, hg * HPG:(hg + 1) * HPG] = r["attnw"]
            fpd += r["fpd"].transpose(1, 0, 2).reshape(C, ND)
            fps += r["fps"].transpose(1, 0, 2).reshape(C, 4 * NS)
        fpd += bp[:, None]
        fps += bp[:, None]
        aD = _zscore(fpd.reshape(C, 32, 32))
        small = []
        shapes = [(8, 24), (8, 24), (24, 8), (24, 8)]
        for j, (hh, ww) in enumerate(shapes):
            blk = fps[:, j * NS:(j + 1) * NS].reshape(C, hh, ww)
            small.append(_zscore(_up2_align_corners(blk)))
        aB1, aB2, aC1, aC2 = small
        attn_all[b, :, 0:16, 0:48] = aB1
        attn_all[b, :, 48:64, 16:64] = aB2
        attn_all[b, :, 16:64, 0:16] = aC1
        attn_all[b, :, 0:48, 48:64] = aC2
        attn_all[b, :, 16:48, 16:48] = aD
    return np.ascontiguousarray(attn_all.transpose(0, 2, 3, 1)), attn_weight


def _run(in_maps, trace=False, **kw):
    from concourse.bass_utils import run_bass_kernel_spmd
    if "nc" not in _CACHED:
        _CACHED["nc"] = _build_nc()
    return run_bass_kernel_spmd(_CACHED["nc"], in_maps, list(range(8)),
                                trace=trace, **kw)


def kernel(x, Cam_Centre, gw_Wqkv, gw_bqkv, as_Wqkv, as_bqkv, as_Wproj, as_bproj):
    in_maps = _prep_inputs(x, gw_Wqkv, gw_bqkv, as_Wqkv, as_bqkv, as_Wproj)
    res = _run(in_maps)
    return _postprocess(res.results, as_bproj)
